# revision 59
# baseline (speedup 1.0000x reference)
"""Trainium2 Bass kernel for nn_NMSquaredGaussianMixture.

Math: output = -(log(sum_n g_n^2) - log z) / N
  g_n = sum_k c_k * exp(E_k(x_n)),  c_k = w_k / sqrt((2pi)^2 det S_k)
  E_k(x) = -0.5 (x-mu_k)^T S_k^{-1} (x-mu_k)
  z     = sum_ij w_i w_j N(mu_i - mu_j; 0, S_i + S_j)   (tiny, host-side)

Device pipeline (per core, data-parallel over samples), per super-tile of
8192 samples (16 groups x 512 = one PSUM bank of free dim):
  E = W1 @ F     features F = [y0^2, y0*y1, y1^2, y0, y1, 1], y = x - ctr
                 (recentered basis keeps the expanded quadratic
                 cancellation-free, so bf16 weights+features hold the
                 end-to-end error at ~4e-3 vs the 2e-2 budget). Two
                 512-col matmuls per super-tile (cluster halves kl=0..7,
                 8..15), E in PSUM [128 parts = 8 clusters x 16 groups].
  dens = exp(E)  split across engines so neither saturates: ACT does an
                 exact exp for cluster-half 0, one [128, 1024] ACTIVATE
                 per super-tile PAIR (amortizes the ~350-cycle fixed
                 cost); DVE does a Schraudolph bf16 exp for half 1
                 (bits = u16(E*128/ln2 + C); the f32->u16 convert
                 saturates negatives to 0 = bf16 +0.0). Separate PSUM E
                 tiles per half keep the two chains independent -- a
                 shared tile makes the tile scheduler chain DVE after ACT.
  g = C^T dens   sign-combine matmuls PSUM-accumulate halves into
                 [16, 512] regions packed 3-pairs-per-bank at partition
                 bases 0/32/64 (quadrant 3 / base 96 is a HW no-go).
  sum g^2        ACT Square with accum_out reduces each full g bank to
                 acc[:, col] in one pass (g dies in PSUM; only [128, 12]
                 of per-bank partial sums ever reach DRAM). Host sums the
                 valid 16-row windows in f64 and applies log / z.

Engine budget per super-tile (~1us steady state): PE 4x 512-col matmuls
(~94% busy, the bottleneck), ACT exp-pair + square share (~87%), DVE
~73%. rhs DMA dispatches alternate between the SP and ACT HWDGE queues
so early chunks land before the pipeline starves; chunk sizes ramp up to
amortize the ~0.6us dispatch + DGE latency per DMA.
"""

import numpy as np

import concourse.mybir as mybir
import concourse.tile as tile
from concourse import bacc
from concourse.bass_utils import run_bass_kernel_spmd

N_SAMPLES = 2_000_000
N_CORES = 8
NC_SAMP = N_SAMPLES // N_CORES  # 250_000
K = 16  # clusters
NF = 5  # features (the constant term folds into the exp bias)
G = 16  # sample groups (one per matmul output column block)
FD = 512  # moving free dim (one PSUM bank of fp32)
SUPER = G * FD  # samples per super-tile = 8192
NST = -(-NC_SAMP // SUPER)  # 31 super-tiles/core
NPAD = NST * SUPER  # 253952 padded samples per core
NPACK = -(-NST // 6)  # g packs of 6 super-tiles (3 bases x 2 banks)
PAD_U = 1.0e6  # pad feature: huge y0^2 --> E << 0 --> dens = 0
# rhs DMA chunk sizes (super-tiles): small first chunks so compute starts
# early, then large chunks to amortize the ~625ns serialized DGE overhead.
RHS_CHUNKS = [2, 2, 2, 2, 3, 4, 4, 6, 6]  # pair-aligned boundaries
assert sum(RHS_CHUNKS) == NST

TWO_PI = 2.0 * np.pi
# bf16-Schraudolph exp constants: bits(bf16 exp(E)) ~= E*128/ln2 + (127*128-C2)
# C2 = 8.0 calibrated end-to-end (cancels the piecewise-linear bias).
SCHRAUD_A2 = float(128.0 / np.log(2.0))
SCHRAUD_B2 = float(127.0 * 128.0 - 8.0)
# pipeline tuning knobs
RHS_BUFS = 9
DA_BUFS = 3
DB_BUFS = 3
E_BUFS = 2
G_BUFS = 2
PIPE_D = 2

_CACHE = {}


def _cluster_params(means, chols, weights):
    """A [K,6] f64 monomial coefficients in a re-centered basis (incl. the
    -0.5 factor and ln|c| const), signs [K], and the center ctr [2].

    Centering at a precision-weighted mean of the cluster means kills the
    catastrophic cancellation that tight clusters (large S^-1) otherwise
    cause in the expanded quadratic, which is what makes bf16 operands
    accurate enough end-to-end."""
    means = np.asarray(means, np.float64)
    chols = np.asarray(chols, np.float64)
    weights = np.asarray(weights, np.float64)
    L = np.tril(chols)
    S = L @ np.swapaxes(L, 1, 2)
    P = np.linalg.inv(S)
    detS = np.linalg.det(S)
    c = weights / np.sqrt(TWO_PI**2 * detS)
    signs = np.where(c >= 0, 1.0, -1.0)
    logc = np.log(np.abs(c))
    pw = np.abs(P).sum(axis=(1, 2))
    ctr = (means * pw[:, None]).sum(0) / pw.sum()
    m = means - ctr[None, :]
    Pm = np.einsum("kij,kj->ki", P, m)
    mPm = np.einsum("ki,ki->k", m, Pm)
    A = np.stack(
        [
            -0.5 * P[:, 0, 0],
            -P[:, 0, 1],
            -0.5 * P[:, 1, 1],
            Pm[:, 0],
            Pm[:, 1],
            -0.5 * mPm + logc,
        ],
        axis=1,
    )
    return A, signs, ctr


def _z_term(means, chols, weights):
    means = np.asarray(means, np.float64)
    chols = np.asarray(chols, np.float64)
    weights = np.asarray(weights, np.float64)
    L = np.tril(chols)
    S = L @ np.swapaxes(L, 1, 2)
    Ssum = S[:, None] + S[None, :]
    mdiff = means[:, None, :] - means[None, :, :]
    m2 = np.einsum("abi,abij,abj->ab", mdiff, np.linalg.inv(Ssum), mdiff)
    Zij = np.exp(-0.5 * m2) / np.sqrt(TWO_PI**2 * np.linalg.det(Ssum))
    return float(np.einsum("i,j,ij->", weights, weights, Zij))


def _build_rhs(X, ctr):
    """X [2M,2] f32 -> per-core rhs [N_CORES, 96, NST*FD] bf16, where
    rhs[c, g*NF+f, st*FD + t] = feat_f of sample
    n = c*NC_SAMP + st*SUPER + g*FD + t  (pad samples give dens == 0)."""
    import ml_dtypes

    X = np.asarray(X, np.float32)
    feats = np.empty((N_CORES, NPAD, NF), np.float32)
    x0 = (X[:, 0] - np.float32(ctr[0])).reshape(N_CORES, NC_SAMP)
    x1 = (X[:, 1] - np.float32(ctr[1])).reshape(N_CORES, NC_SAMP)
    feats[:, :NC_SAMP, 0] = x0 * x0
    feats[:, :NC_SAMP, 1] = x0 * x1
    feats[:, :NC_SAMP, 2] = x1 * x1
    feats[:, :NC_SAMP, 3] = x0
    feats[:, :NC_SAMP, 4] = x1
    feats[:, NC_SAMP:, :] = 0.0
    feats[:, NC_SAMP:, 0] = PAD_U
    # [C, NST, G, FD, NF] -> [C, G, NF, NST, FD] -> [C, 96, NST*FD]
    r = feats.reshape(N_CORES, NST, G, FD, NF).transpose(0, 2, 4, 1, 3)
    return np.ascontiguousarray(r).reshape(N_CORES, G * NF, NST * FD).astype(
        ml_dtypes.bfloat16
    )


def _build_weights(A, signs):
    """w1 [G*NF, 256] bf16 block coefficient mats (cluster halves, quadratic
    features only); cm [128, 2G] bf16 sign-combine mats; bias [128, 2] f32
    per-partition constant terms (col 0: ACT exp bias = A5 of half-0
    cluster; col 1: DVE Schraudolph offset = B2 + A2*A5 of half-1 cluster).
    Out partition m = k_local*G+g."""
    import ml_dtypes

    A32 = A.astype(np.float32)
    w1 = np.zeros((G * NF, 2 * 128), np.float32)
    cm = np.zeros((128, 2 * G), np.float32)
    bias = np.zeros((128, 2), np.float32)
    for half in (0, 1):
        for kl in range(8):
            k = half * 8 + kl
            for g in range(G):
                w1[g * NF : (g + 1) * NF, half * 128 + kl * G + g] = A32[k, :NF]
                cm[kl * G + g, half * G + g] = signs[k]
    for kl in range(8):
        for g in range(G):
            p = kl * G + g
            bias[p, 0] = A[kl, 5]
            bias[p, 1] = SCHRAUD_B2 + SCHRAUD_A2 * A[8 + kl, 5]
    return w1.astype(ml_dtypes.bfloat16), cm.astype(ml_dtypes.bfloat16), bias


def _build_bass():
    nc = bacc.Bacc("TRN2", target_bir_lowering=False, debug=False)
    f32 = mybir.dt.float32
    bf16 = mybir.dt.bfloat16
    u16 = mybir.dt.uint16
    rhs_d = nc.dram_tensor("rhs", [G * NF, NST * FD], bf16, kind="ExternalInput")
    w1_d = nc.dram_tensor("w1", [G * NF, 2 * 128], bf16, kind="ExternalInput")
    cm_d = nc.dram_tensor("cm", [128, 2 * G], bf16, kind="ExternalInput")
    bias_d = nc.dram_tensor("bias", [128, 2], f32, kind="ExternalInput")
    gout_d = nc.dram_tensor("gout", [128, 2 * NPACK], f32, kind="ExternalOutput")

    with tile.TileContext(nc) as tc:
        with (
            tc.tile_pool(name="const", bufs=1) as cpool,
            tc.tile_pool(name="rhs", bufs=RHS_BUFS) as rpool,
            tc.tile_pool(name="densa", bufs=DA_BUFS) as dapool,
            tc.tile_pool(name="densb", bufs=DB_BUFS) as dbpool,
            tc.tile_pool(name="pea", bufs=E_BUFS, space="PSUM") as eapool,
            tc.tile_pool(name="peb", bufs=E_BUFS, space="PSUM") as ebpool,
            tc.tile_pool(name="pg", bufs=G_BUFS, space="PSUM") as gpool,
        ):
            # PE warm-up on a memset tile: keeps the HAM clock-gate from
            # throttling the first real matmuls while the w1/rhs DMAs are
            # still in flight.
            wz = cpool.tile([128, 128], bf16)
            nc.gpsimd.memset(wz[:], 0)
            warm = gpool.tile([128, 128], f32, tag="g")
            for _ in range(10):
                nc.tensor.matmul(
                    warm[:], wz[:], wz[:], start=True, stop=True
                )

            w1 = cpool.tile([G * NF, 2 * 128], bf16)
            cm = cpool.tile([128, 2 * G], bf16)
            acc = cpool.tile([128, 2 * NPACK], f32)

            # rhs loads alternate between the two HWDGE queues (SP + ACT)
            # so dispatch + descriptor-gen for early chunks parallelize;
            # ACT is idle until the first exp anyway.
            biast = cpool.tile([128, 2], f32)
            nc.scalar.dma_start(w1[:], w1_d[:])
            nc.scalar.dma_start(biast[:], bias_d[:])
            rhs_views = {}  # st -> (chunk tile, col offset)
            lo = 0
            for ci, sz in enumerate(RHS_CHUNKS):
                hi = lo + sz
                rt = rpool.tile([G * NF, max(RHS_CHUNKS) * FD], bf16, tag="rhs")
                eng = nc.sync if ci % 2 == 0 else nc.scalar
                eng.dma_start(rt[:, : sz * FD], rhs_d[:, lo * FD : hi * FD])
                if ci == 2:
                    nc.sync.dma_start(cm[:], cm_d[:])
                for st in range(lo, hi):
                    rhs_views[st] = (rt, (st - lo) * FD)
                lo = hi

            # Software pipeline at super-tile-PAIR granularity: rhs chunks
            # are pair-aligned so mm1a / mm2 run as single 1024-col matmuls
            # per pair (half the PE instruction + LDWEIGHTS count). The
            # sign-combine stage is delayed by D pairs so PE's in-order
            # stream never blocks on the exp engines.
            D = PIPE_D
            NPAIR = -(-NST // 2)  # 16, last pair is st30 alone
            dens_ring = [None] * NPAIR
            g_hold = [None]

            pair_state = {}  # p -> (ea, da, db, w)

            def emit_front_half(p, h):
                st = 2 * p + h
                if st >= NST:
                    return
                if h == 0:
                    full = st + 1 < NST
                    w = 2 * FD if full else FD
                    # E tiles are separate per cluster-half so the ACT and
                    # DVE exp chains stay independent (a shared tile makes
                    # the scheduler serialize DVE behind ACT).
                    ea = eapool.tile([128, 2 * FD], f32, tag="ea", name=f"ea{p}")
                    da = dapool.tile(
                        [128, 2 * FD], bf16, tag="densa", name=f"da{p}"
                    )
                    db = dbpool.tile(
                        [128, 2 * FD], u16, tag="densb", name=f"db{p}"
                    )
                    pair_state[p] = (ea, da, db, w)
                ea, da, db, w = pair_state[p]
                rt, lo = rhs_views[st]
                rhs = rt[:, lo : lo + FD]
                nc.tensor.matmul(
                    ea[:, h * FD : (h + 1) * FD], w1[:, 0:128], rhs,
                    start=True, stop=True,
                )
                ebh = ebpool.tile([128, FD], f32, tag="eb", name=f"eb{p}_{h}")
                nc.tensor.matmul(ebh[:], w1[:, 128:256], rhs, start=True, stop=True)
                # DVE half: Schraudolph bf16 exp (the f32->u16 convert
                # saturates negatives to 0 == bf16 +0.0, so the underflow
                # band needs no clamp; ~1% sawtooth error washes out over
                # the 2M-sample reduction).
                nc.vector.tensor_scalar(
                    db[:, h * FD : (h + 1) * FD],
                    ebh[:],
                    SCHRAUD_A2,
                    biast[:, 1:2],
                    op0=mybir.AluOpType.mult,
                    op1=mybir.AluOpType.add,
                )
                if h + 1 == w // FD:
                    # one exact exp on ACT for the pair's half0 energies
                    nc.scalar.activation(
                        da[:, 0:w],
                        ea[:, 0:w],
                        mybir.ActivationFunctionType.Exp,
                        bias=biast[:, 0:1],
                    )
                    dens_ring[p] = pair_state.pop(p)

            def emit_back_half(p, h):
                st = 2 * p + h
                if st >= NST:
                    return
                _, da, db, w = dens_ring[p]
                # 3 pairs per g pack at partition bases 0/32/64 (base 96 =
                # quadrant 3 is a HW no-go); pair halves land in two
                # separate single-bank g tiles so each can be squared (and
                # its buffer recycled) as soon as its 3 slots are full.
                # h0/h1 cluster-halves accumulate in PSUM per slot.
                pk, sp = p // 3, p % 3
                last = p == NPAIR - 1
                if sp == 0 and h == 0:
                    nh = 2 if (not last or w == 2 * FD) else 1
                    g_hold[0] = [
                        gpool.tile([128, FD], f32, tag="g", name=f"gt{pk}_{q}")
                        for q in range(nh)
                    ]
                gs = g_hold[0]
                pbase = 32 * sp
                region = gs[h][pbase : pbase + G, :]
                sl = slice(h * FD, (h + 1) * FD)
                nc.tensor.matmul(
                    region, cm[:, 0:G], da[:, sl],
                    start=True, stop=False, skip_group_check=True,
                )
                nc.tensor.matmul(
                    region, cm[:, G : 2 * G], db[:, sl].bitcast(bf16),
                    start=False, stop=True, skip_group_check=True,
                )
                if sp == 2:
                    # half-pack complete: sum-of-squares, in place
                    nc.scalar.activation(
                        gs[h][:],
                        gs[h][:],
                        mybir.ActivationFunctionType.Square,
                        accum_out=acc[:, 2 * pk + h : 2 * pk + h + 1],
                    )
                    if h == 1:
                        # stream the pack's acc columns out now so the final
                        # store only waits on the last pack
                        nc.sync.dma_start(
                            gout_d[:, 2 * pk : 2 * pk + 2],
                            acc[:, 2 * pk : 2 * pk + 2],
                        )
                if last and h + 1 == w // FD and sp != 2:
                    # partial pack: per-pair-region squares
                    for q in range(3 * pk, NPAIR):
                        qb = 32 * (q % 3)
                        for qh in range((2 * FD if 2 * q + 1 < NST else FD) // FD):
                            rg = gs[qh][qb : qb + G, :]
                            nc.scalar.activation(
                                rg,
                                rg,
                                mybir.ActivationFunctionType.Square,
                                accum_out=acc[qb : qb + G, 2 * pk + qh : 2 * pk + qh + 1],
                            )
                    nc.sync.dma_start(
                        gout_d[:, 2 * pk :], acc[:, 2 * pk :]
                    )

            for p in range(NPAIR + D):
                if p < NPAIR:
                    emit_front_half(p, 0)
                    emit_front_half(p, 1)
                if p >= D:
                    emit_back_half(p - D, 0)
                    emit_back_half(p - D, 1)

    nc.compile()
    return nc


def _get_bass():
    if "nc" not in _CACHE:
        _CACHE["nc"] = _build_bass()
    return _CACHE["nc"]


def kernel(X, means, chols, weights, it=None, **_unused):
    X = np.ascontiguousarray(np.asarray(X, np.float32))
    assert X.shape == (N_SAMPLES, 2), X.shape

    A, signs, ctr = _cluster_params(means, chols, weights)
    A32 = A.astype(np.float32)
    z = _z_term(means, chols, weights)

    w1, cm, bias = _build_weights(A, signs)
    rhs = _build_rhs(X, ctr)

    nc = _get_bass()
    in_maps = [
        {"rhs": rhs[c], "w1": w1, "cm": cm, "bias": bias} for c in range(N_CORES)
    ]
    res = run_bass_kernel_spmd(nc, in_maps, core_ids=list(range(N_CORES)))

    total = 0.0
    for r in res.results:
        go = r["gout"].astype(np.float64)  # [128, 2*NPACK] half-pack sum(g^2)
        # acc[32*(p%3) + 0:16, 2*(p//3) + h] holds the sum-of-squares for
        # super-tile 2p+h; other rows are garbage.
        npair = -(-NST // 2)
        for p in range(npair):
            pb = 32 * (p % 3)
            for h in range(2 if 2 * p + 1 < NST else 1):
                total += float(go[pb : pb + G, 2 * (p // 3) + h].sum())

    out = -(np.log(total) - np.log(z)) / N_SAMPLES
    return np.float32(out)


if __name__ == "__main__":
    rng = np.random.default_rng(0)
    X = rng.standard_normal((N_SAMPLES, 2), dtype=np.float32)
    scale = 2.0 * (1.0 + rng.standard_normal((K, 1, 1), dtype=np.float32))
    chols = scale * np.ones((2, 2), np.float32)[None] + 0.5 * np.eye(2, dtype=np.float32)[None]
    means = rng.standard_normal((K, 2), dtype=np.float32)
    weights = rng.standard_normal(K, dtype=np.float32)
    print(kernel(X, means, chols, weights, 1))


# revision 62
# speedup vs baseline: 1.1129x; 1.1129x over previous
"""Trainium2 Bass kernel for nn_NMSquaredGaussianMixture.

Math: output = -(log(sum_n g_n^2) - log z) / N
  g_n = sum_k c_k * exp(E_k(x_n)),  c_k = w_k / sqrt((2pi)^2 det S_k)
  E_k(x) = -0.5 (x-mu_k)^T S_k^{-1} (x-mu_k)
  z     = sum_ij w_i w_j N(mu_i - mu_j; 0, S_i + S_j)   (tiny, host-side)

Device pipeline (per core, data-parallel over samples), per super-tile of
8192 samples (16 groups x 512 = one PSUM bank of free dim):
  E = W1 @ F     features F = [y0^2, y0*y1, y1^2, y0, y1, 1], y = x - ctr
                 (recentered basis keeps the expanded quadratic
                 cancellation-free, so bf16 weights+features hold the
                 end-to-end error at ~4e-3 vs the 2e-2 budget). Two
                 512-col matmuls per super-tile (cluster halves kl=0..7,
                 8..15), E in PSUM [128 parts = 8 clusters x 16 groups].
  dens = exp(E)  split across engines so neither saturates: ACT does an
                 exact exp for cluster-half 0, one [128, 1024] ACTIVATE
                 per super-tile PAIR (amortizes the ~350-cycle fixed
                 cost); DVE does a Schraudolph bf16 exp for half 1
                 (bits = u16(E*128/ln2 + C); the f32->u16 convert
                 saturates negatives to 0 = bf16 +0.0). Separate PSUM E
                 tiles per half keep the two chains independent -- a
                 shared tile makes the tile scheduler chain DVE after ACT.
  g = C^T dens   sign-combine matmuls PSUM-accumulate halves into
                 [16, 512] regions packed 3-pairs-per-bank at partition
                 bases 0/32/64 (quadrant 3 / base 96 is a HW no-go).
  sum g^2        ACT Square with accum_out reduces each full g bank to
                 acc[:, col] in one pass (g dies in PSUM; only [128, 12]
                 of per-bank partial sums ever reach DRAM). Host sums the
                 valid 16-row windows in f64 and applies log / z.

Engine budget per super-tile (~1us steady state): PE 4x 512-col matmuls
(~94% busy, the bottleneck), ACT exp-pair + square share (~87%), DVE
~73%. rhs DMA dispatches alternate between the SP and ACT HWDGE queues
so early chunks land before the pipeline starves; chunk sizes ramp up to
amortize the ~0.6us dispatch + DGE latency per DMA.
"""

import numpy as np

import concourse.mybir as mybir
import concourse.tile as tile
from concourse import bacc
from concourse.bass_utils import run_bass_kernel_spmd

N_SAMPLES = 2_000_000
N_CORES = 8
NC_SAMP = N_SAMPLES // N_CORES  # 250_000
K = 16  # clusters
NF = 5  # features (the constant term folds into the exp bias)
G = 16  # sample groups (one per matmul output column block)
FD = 512  # moving free dim (one PSUM bank of fp32)
SUPER = G * FD  # samples per super-tile = 8192
NST = -(-NC_SAMP // SUPER)  # 31 super-tiles/core
NPAD = NST * SUPER  # 253952 padded samples per core
NPACK = -(-NST // 6)  # g packs of 6 super-tiles (3 bases x 2 banks)
PAD_U = 1.0e6  # pad feature: huge y0^2 --> E << 0 --> dens = 0
# rhs DMA chunk sizes (super-tiles): small first chunks so compute starts
# early, then large chunks to amortize the ~625ns serialized DGE overhead.
RHS_CHUNKS = [2, 2, 2, 2, 3, 4, 4, 6, 6]  # pair-aligned boundaries
assert sum(RHS_CHUNKS) == NST

TWO_PI = 2.0 * np.pi
# bf16-Schraudolph exp constants: bits(bf16 exp(E)) ~= E*128/ln2 + (127*128-C2)
# C2 = 8.0 calibrated end-to-end (cancels the piecewise-linear bias).
SCHRAUD_A2 = float(128.0 / np.log(2.0))
SCHRAUD_B2 = float(127.0 * 128.0 - 8.0)
# pipeline tuning knobs
RHS_BUFS = 9
DA_BUFS = 3
DB_BUFS = 3
E_BUFS = 2
G_BUFS = 2
PIPE_D = 2

_CACHE = {}


def _cluster_params(means, chols, weights):
    """A [K,6] f64 monomial coefficients in a re-centered basis (incl. the
    -0.5 factor and ln|c| const), signs [K], and the center ctr [2].

    Centering at a precision-weighted mean of the cluster means kills the
    catastrophic cancellation that tight clusters (large S^-1) otherwise
    cause in the expanded quadratic, which is what makes bf16 operands
    accurate enough end-to-end."""
    means = np.asarray(means, np.float64)
    chols = np.asarray(chols, np.float64)
    weights = np.asarray(weights, np.float64)
    L = np.tril(chols)
    S = L @ np.swapaxes(L, 1, 2)
    P = np.linalg.inv(S)
    detS = np.linalg.det(S)
    c = weights / np.sqrt(TWO_PI**2 * detS)
    signs = np.where(c >= 0, 1.0, -1.0)
    logc = np.log(np.abs(c))
    pw = np.abs(P).sum(axis=(1, 2))
    ctr = (means * pw[:, None]).sum(0) / pw.sum()
    m = means - ctr[None, :]
    Pm = np.einsum("kij,kj->ki", P, m)
    mPm = np.einsum("ki,ki->k", m, Pm)
    A = np.stack(
        [
            -0.5 * P[:, 0, 0],
            -P[:, 0, 1],
            -0.5 * P[:, 1, 1],
            Pm[:, 0],
            Pm[:, 1],
            -0.5 * mPm + logc,
        ],
        axis=1,
    )
    return A, signs, ctr


def _z_term(means, chols, weights):
    means = np.asarray(means, np.float64)
    chols = np.asarray(chols, np.float64)
    weights = np.asarray(weights, np.float64)
    L = np.tril(chols)
    S = L @ np.swapaxes(L, 1, 2)
    Ssum = S[:, None] + S[None, :]
    mdiff = means[:, None, :] - means[None, :, :]
    m2 = np.einsum("abi,abij,abj->ab", mdiff, np.linalg.inv(Ssum), mdiff)
    Zij = np.exp(-0.5 * m2) / np.sqrt(TWO_PI**2 * np.linalg.det(Ssum))
    return float(np.einsum("i,j,ij->", weights, weights, Zij))


def _build_rhs(X, ctr):
    """X [2M,2] f32 -> per-core rhs [N_CORES, 96, NST*FD] bf16, where
    rhs[c, g*NF+f, st*FD + t] = feat_f of sample
    n = c*NC_SAMP + st*SUPER + g*FD + t  (pad samples give dens == 0)."""
    import ml_dtypes

    X = np.asarray(X, np.float32)
    feats = np.empty((N_CORES, NPAD, NF), np.float32)
    x0 = (X[:, 0] - np.float32(ctr[0])).reshape(N_CORES, NC_SAMP)
    x1 = (X[:, 1] - np.float32(ctr[1])).reshape(N_CORES, NC_SAMP)
    feats[:, :NC_SAMP, 0] = x0 * x0
    feats[:, :NC_SAMP, 1] = x0 * x1
    feats[:, :NC_SAMP, 2] = x1 * x1
    feats[:, :NC_SAMP, 3] = x0
    feats[:, :NC_SAMP, 4] = x1
    feats[:, NC_SAMP:, :] = 0.0
    feats[:, NC_SAMP:, 0] = PAD_U
    # [C, NST, G, FD, NF] -> [C, G, NF, NST, FD] -> [C, 96, NST*FD]
    r = feats.reshape(N_CORES, NST, G, FD, NF).transpose(0, 2, 4, 1, 3)
    return np.ascontiguousarray(r).reshape(N_CORES, G * NF, NST * FD).astype(
        ml_dtypes.bfloat16
    )


def _build_weights(A, signs):
    """w1 [G*NF, 256] bf16 block coefficient mats (cluster halves, quadratic
    features only); cm [128, 2G] bf16 sign-combine mats; bias [128, 2] f32
    per-partition constant terms (col 0: ACT exp bias = A5 of half-0
    cluster; col 1: DVE Schraudolph offset = B2 + A2*A5 of half-1 cluster).
    Out partition m = k_local*G+g."""
    import ml_dtypes

    A32 = A.astype(np.float32)
    w1 = np.zeros((G * NF, 2 * 128), np.float32)
    cm = np.zeros((128, 2 * G), np.float32)
    bias = np.zeros((128, 2), np.float32)
    for half in (0, 1):
        for kl in range(8):
            k = half * 8 + kl
            for g in range(G):
                w1[g * NF : (g + 1) * NF, half * 128 + kl * G + g] = A32[k, :NF]
                cm[kl * G + g, half * G + g] = signs[k]
    for kl in range(8):
        for g in range(G):
            p = kl * G + g
            bias[p, 0] = A[kl, 5]
            bias[p, 1] = SCHRAUD_B2 + SCHRAUD_A2 * A[8 + kl, 5]
    return w1.astype(ml_dtypes.bfloat16), cm.astype(ml_dtypes.bfloat16), bias


def _build_bass():
    nc = bacc.Bacc("TRN2", target_bir_lowering=False, debug=False)
    f32 = mybir.dt.float32
    bf16 = mybir.dt.bfloat16
    u16 = mybir.dt.uint16
    rhs_d = nc.dram_tensor("rhs", [G * NF, NST * FD], bf16, kind="ExternalInput")
    w1_d = nc.dram_tensor("w1", [G * NF, 2 * 128], bf16, kind="ExternalInput")
    cm_d = nc.dram_tensor("cm", [128, 2 * G], bf16, kind="ExternalInput")
    bias_d = nc.dram_tensor("bias", [128, 2], f32, kind="ExternalInput")
    gout_d = nc.dram_tensor("gout", [128, 2 * NPACK], f32, kind="ExternalOutput")

    with tile.TileContext(nc) as tc:
        with (
            tc.tile_pool(name="const", bufs=1) as cpool,
            tc.tile_pool(name="rhs", bufs=RHS_BUFS) as rpool,
            tc.tile_pool(name="densa", bufs=DA_BUFS) as dapool,
            tc.tile_pool(name="densb", bufs=DB_BUFS) as dbpool,
            tc.tile_pool(name="pea", bufs=E_BUFS, space="PSUM") as eapool,
            tc.tile_pool(name="peb", bufs=E_BUFS, space="PSUM") as ebpool,
            tc.tile_pool(name="pg", bufs=G_BUFS, space="PSUM") as gpool,
        ):
            # PE warm-up on a memset tile: keeps the HAM clock-gate from
            # throttling the first real matmuls while the w1/rhs DMAs are
            # still in flight.
            wz = cpool.tile([128, 128], bf16)
            nc.gpsimd.memset(wz[:], 0)
            warm = gpool.tile([128, 128], f32, tag="g")
            for _ in range(10):
                nc.tensor.matmul(
                    warm[:], wz[:], wz[:], start=True, stop=True
                )

            w1 = cpool.tile([G * NF, 2 * 128], bf16)
            cm = cpool.tile([128, 2 * G], bf16)
            acc = cpool.tile([128, 2 * NPACK], f32)

            # rhs loads alternate between the two HWDGE queues (SP + ACT)
            # so dispatch + descriptor-gen for early chunks parallelize;
            # ACT is idle until the first exp anyway.
            biast = cpool.tile([128, 2], f32)
            nc.scalar.dma_start(w1[:], w1_d[:])
            nc.scalar.dma_start(biast[:], bias_d[:])
            rhs_views = {}  # st -> (chunk tile, col offset)
            lo = 0
            for ci, sz in enumerate(RHS_CHUNKS):
                hi = lo + sz
                rt = rpool.tile([G * NF, max(RHS_CHUNKS) * FD], bf16, tag="rhs")
                eng = nc.sync if ci % 2 == 0 else nc.scalar
                eng.dma_start(rt[:, : sz * FD], rhs_d[:, lo * FD : hi * FD])
                if ci == 2:
                    nc.sync.dma_start(cm[:], cm_d[:])
                for st in range(lo, hi):
                    rhs_views[st] = (rt, (st - lo) * FD)
                lo = hi

            # Software pipeline at super-tile-PAIR granularity: rhs chunks
            # are pair-aligned so mm1a / mm2 run as single 1024-col matmuls
            # per pair (half the PE instruction + LDWEIGHTS count). The
            # sign-combine stage is delayed by D pairs so PE's in-order
            # stream never blocks on the exp engines.
            D = PIPE_D
            NPAIR = -(-NST // 2)  # 16, last pair is st30 alone
            dens_ring = [None] * NPAIR
            g_hold = [None]

            pair_state = {}  # p -> (ea, da, db, w)

            def emit_front_half(p, h):
                st = 2 * p + h
                if st >= NST:
                    return
                if h == 0:
                    full = st + 1 < NST
                    w = 2 * FD if full else FD
                    # E tiles are separate per cluster-half so the ACT and
                    # DVE exp chains stay independent (a shared tile makes
                    # the scheduler serialize DVE behind ACT).
                    ea = eapool.tile([128, 2 * FD], f32, tag="ea", name=f"ea{p}")
                    da = dapool.tile(
                        [128, 2 * FD], bf16, tag="densa", name=f"da{p}"
                    )
                    db = dbpool.tile(
                        [128, 2 * FD], u16, tag="densb", name=f"db{p}"
                    )
                    pair_state[p] = (ea, da, db, w)
                ea, da, db, w = pair_state[p]
                rt, lo = rhs_views[st]
                rhs = rt[:, lo : lo + FD]
                nc.tensor.matmul(
                    ea[:, h * FD : (h + 1) * FD], w1[:, 0:128], rhs,
                    start=True, stop=True,
                )
                ebh = ebpool.tile([128, FD], f32, tag="eb", name=f"eb{p}_{h}")
                nc.tensor.matmul(ebh[:], w1[:, 128:256], rhs, start=True, stop=True)
                # DVE half: Schraudolph bf16 exp (the f32->u16 convert
                # saturates negatives to 0 == bf16 +0.0, so the underflow
                # band needs no clamp; ~1% sawtooth error washes out over
                # the 2M-sample reduction).
                nc.vector.tensor_scalar(
                    db[:, h * FD : (h + 1) * FD],
                    ebh[:],
                    SCHRAUD_A2,
                    biast[:, 1:2],
                    op0=mybir.AluOpType.mult,
                    op1=mybir.AluOpType.add,
                )
                if h + 1 == w // FD:
                    # one exact exp on ACT for the pair's half0 energies
                    nc.scalar.activation(
                        da[:, 0:w],
                        ea[:, 0:w],
                        mybir.ActivationFunctionType.Exp,
                        bias=biast[:, 0:1],
                    )
                    dens_ring[p] = pair_state.pop(p)

            def emit_back_half(p, h):
                st = 2 * p + h
                if st >= NST:
                    return
                _, da, db, w = dens_ring[p]
                # 3 pairs (6 super-tile regions) per g pack at partition
                # bases 0/32/64 (base 96 = quadrant 3 is a HW no-go), in two
                # single-bank g tiles filled in REGION order q = 2*sp + h:
                # tile q//3, base 32*(q%3). A tile fills after 1.5 pairs, so
                # its square lands mid-pack (smooth ACT load) and its buffer
                # recycles early. h0/h1 cluster-halves accumulate in PSUM.
                pk, sp = p // 3, p % 3
                last = p == NPAIR - 1
                q = 2 * sp + h
                if q == 0:
                    nq = 2 * (3 - sp) if not last else (w // FD)
                    g_hold[0] = [
                        gpool.tile([128, FD], f32, tag="g", name=f"gt{pk}_{t}")
                        for t in range(-(-nq // 3))
                    ]
                gs = g_hold[0]
                region = gs[q // 3][32 * (q % 3) : 32 * (q % 3) + G, :]
                sl = slice(h * FD, (h + 1) * FD)
                nc.tensor.matmul(
                    region, cm[:, 0:G], da[:, sl],
                    start=True, stop=False, skip_group_check=True,
                )
                nc.tensor.matmul(
                    region, cm[:, G : 2 * G], db[:, sl].bitcast(bf16),
                    start=False, stop=True, skip_group_check=True,
                )
                if q in (2, 5):
                    # g tile full: one sum-of-squares pass, in place
                    t = q // 3
                    nc.scalar.activation(
                        gs[t][:],
                        gs[t][:],
                        mybir.ActivationFunctionType.Square,
                        accum_out=acc[:, 2 * pk + t : 2 * pk + t + 1],
                    )
                    if q == 5:
                        # stream the pack's acc columns out now so the final
                        # store only waits on the last pack
                        nc.sync.dma_start(
                            gout_d[:, 2 * pk : 2 * pk + 2],
                            acc[:, 2 * pk : 2 * pk + 2],
                        )
                elif last and h + 1 == w // FD:
                    # partial pack tail: square the unfinished tile (tiles
                    # whose 3 regions filled were already squared at q==2/5)
                    t = q // 3
                    for r in range(q % 3 + 1):
                        rg = gs[t][32 * r : 32 * r + G, :]
                        nc.scalar.activation(
                            rg,
                            rg,
                            mybir.ActivationFunctionType.Square,
                            accum_out=acc[32 * r : 32 * r + G, 2 * pk + t : 2 * pk + t + 1],
                        )
                    nc.sync.dma_start(gout_d[:, 2 * pk :], acc[:, 2 * pk :])

            for p in range(NPAIR + D):
                if p < NPAIR:
                    emit_front_half(p, 0)
                    emit_front_half(p, 1)
                if p >= D:
                    emit_back_half(p - D, 0)
                    emit_back_half(p - D, 1)

    nc.compile()
    return nc


def _get_bass():
    if "nc" not in _CACHE:
        _CACHE["nc"] = _build_bass()
    return _CACHE["nc"]


def kernel(X, means, chols, weights, it=None, **_unused):
    X = np.ascontiguousarray(np.asarray(X, np.float32))
    assert X.shape == (N_SAMPLES, 2), X.shape

    A, signs, ctr = _cluster_params(means, chols, weights)
    A32 = A.astype(np.float32)
    z = _z_term(means, chols, weights)

    w1, cm, bias = _build_weights(A, signs)
    rhs = _build_rhs(X, ctr)

    nc = _get_bass()
    in_maps = [
        {"rhs": rhs[c], "w1": w1, "cm": cm, "bias": bias} for c in range(N_CORES)
    ]
    res = run_bass_kernel_spmd(nc, in_maps, core_ids=list(range(N_CORES)))

    total = 0.0
    for r in res.results:
        go = r["gout"].astype(np.float64)  # [128, 2*NPACK] per-tile sum(g^2)
        # super-tile 2p+h: region q = 2*(p%3)+h -> acc[32*(q%3) + 0:16,
        # 2*(p//3) + q//3]; other rows are garbage.
        npair = -(-NST // 2)
        for p in range(npair):
            for h in range(2 if 2 * p + 1 < NST else 1):
                qr = 2 * (p % 3) + h
                pb = 32 * (qr % 3)
                total += float(go[pb : pb + G, 2 * (p // 3) + qr // 3].sum())

    out = -(np.log(total) - np.log(z)) / N_SAMPLES
    return np.float32(out)


if __name__ == "__main__":
    rng = np.random.default_rng(0)
    X = rng.standard_normal((N_SAMPLES, 2), dtype=np.float32)
    scale = 2.0 * (1.0 + rng.standard_normal((K, 1, 1), dtype=np.float32))
    chols = scale * np.ones((2, 2), np.float32)[None] + 0.5 * np.eye(2, dtype=np.float32)[None]
    means = rng.standard_normal((K, 2), dtype=np.float32)
    weights = rng.standard_normal(K, dtype=np.float32)
    print(kernel(X, means, chols, weights, 1))
